# revision 25
# baseline (speedup 1.0000x reference)
"""Trainium2 Bass kernel for: out = A @ dequant_int4(weight, weight_scale) + bias.

Problem shapes (fp32 A, packed-int4 weight):
    A            [8192, 4096] f32
    weight       [2048, 11008] u8   (two int4 nibbles per byte along K;
                                     row 2i = low nibble, row 2i+1 = high nibble)
    weight_scale [128, 11008] f32   (per-group scale, group_size=32 along K)
    bias         [11008] f32
    out          [8192, 11008] f32

Sharding: tensor-parallel along out_features N across 8 NeuronCores.
Each core gets the full A, a 1376-wide column slice of weight/scale/bias and
computes its [8192, 1376] output slice; the host concatenates slices.

Per-core kernel strategy (v3):
  - A is transposed + cast to bf16 on the host once (shared by all cores)
    and shipped as AT[16, 128, 2, 8192] where element [b, p, t, m] is
    A[m, 256b + 2p + t] — exactly the k-on-partitions order the nibble
    unpack produces.  No PE transposes.
  - Dequantize the full weight slice once into resident SBUF
    ([128, 32, 1376] bf16, k2 on partitions), chunk-major so the first
    512-wide n-chunk is ready early.  Packed bytes and scales arrive per
    chunk as single slab DMAs ([128, 16, 512]); DVE and Pool split the
    6-op unpack per packed k-block.  Scales are host-replicated per
    packed row and pre-cast to bf16.
  - Main loop: per 256 m-columns one wide DMA set loads the A^T tiles
    (512B descriptors) on the ACT ring; per 128-row m-chunk and n-chunk,
    32 matmuls accumulate into one PSUM bank; DVE adds bias during the
    PSUM->SBUF eviction and the chunk is stored straight from SBUF.
"""

import numpy as np
from ml_dtypes import bfloat16

import concourse.bacc as bacc
import concourse.bass as bass
import concourse.tile as tile
from concourse import mybir
from concourse.bass_utils import run_bass_kernel_spmd

M, K, N = 8192, 4096, 11008
NCORES = 8
NS = N // NCORES  # 1376 out-features per core
K2 = K // 2       # 2048 packed rows
P = 128
NB2 = K2 // P     # 16 packed k-blocks
NKB = K // P      # 32 unpacked k-blocks
MW = 256          # m-columns per At DMA group (2 m-chunks)


def _n_chunks(ns, step=512):
    out = []
    n0 = 0
    while n0 < ns:
        out.append((n0, min(step, ns - n0)))
        n0 += step
    return out


def build_nc(m=M, ns=NS, debug=False, reps=1, psum_bufs=4, at_bufs=2, o_bufs=3,
             dq_mode="dve_act_pool", _skip_dequant=False, _skip_main=False):
    """Build the per-core Bass program (identical on all cores).

    reps>1 repeats the whole computation back-to-back inside one NEFF —
    used only by the timing harness to measure steady-state HW time per
    execution with dispatch overhead cancelled.
    """
    n_chunks = _n_chunks(ns)
    mg = m // MW  # At DMA groups
    mpg = MW // P  # m-chunks per group

    # Bacc (not raw Bass): its compile() legalizes multi-semaphore waits into
    # the single event slot each DMA/engine instruction has in the ISA.
    nc = bacc.Bacc(None, target_bir_lowering=False, debug=debug)
    AT = nc.dram_tensor("at", [NKB, P, m], mybir.dt.bfloat16, kind="ExternalInput")
    WQ = nc.dram_tensor("wq", [K2, ns], mybir.dt.uint8, kind="ExternalInput")
    SREP = nc.dram_tensor("srep", [K2, ns], mybir.dt.bfloat16, kind="ExternalInput")
    BIAS = nc.dram_tensor("bias", [P, ns], mybir.dt.float32, kind="ExternalInput")
    OUT = nc.dram_tensor("out", [m, ns], mybir.dt.float32, kind="ExternalOutput")
    # [p, b, n] view: packed row 128b + p
    WQ_V = WQ.rearrange("(b p) n -> p b n", b=NB2, p=P)
    SREP_V = SREP.rearrange("(b p) n -> p b n", b=NB2, p=P)
    # [p, kb, m] view of A^T so a whole At group loads in ONE DMA
    AT_V = AT.rearrange("kb p m -> p kb m")

    with tile.TileContext(nc) as tc:
        with (
            tc.tile_pool(name="singles", bufs=1) as singles,
            tc.tile_pool(name="wpool", bufs=1) as wpool,
            tc.tile_pool(name="slab", bufs=2) as slab,
            tc.tile_pool(name="dq", bufs=3) as dq,
            tc.tile_pool(name="atpool", bufs=at_bufs) as atpool,
            tc.tile_pool(name="opool", bufs=o_bufs) as opool,
            tc.tile_pool(name="psum_o", bufs=psum_bufs, space="PSUM") as psum_o,
        ):
          def body():
            # bias arrives host-replicated to [P, ns]: a stride-0 broadcast
            # DMA trips walrus codegen ("Too many sync wait commands")
            bias_t = singles.tile([P, ns], mybir.dt.float32)
            nc.sync.dma_start(out=bias_t, in_=BIAS[:, :])

            # ---- one-shot dequant of the weight slice into resident SBUF ----
            # chunk-major so n-chunk 0 is fully dequantized first; DVE and
            # Pool split the per-block unpack (Pool gets more: it is faster
            # and DVE also handles the main-loop evictions).
            wsb = wpool.tile([P, NKB, ns], mybir.dt.bfloat16)
            for (n0, nch) in (() if _skip_dequant else tuple(n_chunks)):
                pk = slab.tile([P, NB2, 512], mybir.dt.uint8, tag="pk")
                nc.gpsimd.dma_start(out=pk[:, :, :nch], in_=WQ_V[:, :, n0:n0 + nch])
                st = slab.tile([P, NB2, 512], mybir.dt.bfloat16, tag="st")
                nc.gpsimd.dma_start(out=st[:, :, :nch], in_=SREP_V[:, :, n0:n0 + nch])
                for b in range(NB2):
                    # Pool's ISA has no bitwise ops, so the u8 nibble
                    # extracts live on DVE.  walrus requires each
                    # tensor_scalar's ops to be a single ISA-supported class,
                    # so extract and subtract can't fuse; the subtract can
                    # ride ACT's activation (out = Copy(in*1 + (-8))) which
                    # also does the u8->bf16 cast.  Pool's Q7 software
                    # tensor ops measured ~3-5us per [128,512] tile — avoid.
                    lo = dq.tile([P, 512], mybir.dt.bfloat16, tag="lo")
                    hi = dq.tile([P, 512], mybir.dt.bfloat16, tag="hi")
                    lq = dq.tile([P, 512], mybir.dt.uint8, tag="lq")
                    hq = dq.tile([P, 512], mybir.dt.uint8, tag="hq")
                    nc.vector.tensor_scalar(
                        out=lq[:, :nch], in0=pk[:, b, :nch], scalar1=15, scalar2=None,
                        op0=mybir.AluOpType.bitwise_and)
                    nc.vector.tensor_scalar(
                        out=hq[:, :nch], in0=pk[:, b, :nch], scalar1=4, scalar2=None,
                        op0=mybir.AluOpType.logical_shift_right)
                    if dq_mode in ("dve_act", "dve_act_pool"):
                        nc.scalar.activation(
                            out=lo[:, :nch], in_=lq[:, :nch],
                            func=mybir.ActivationFunctionType.Copy,
                            bias=-8.0, scale=1.0)
                        nc.scalar.activation(
                            out=hi[:, :nch], in_=hq[:, :nch],
                            func=mybir.ActivationFunctionType.Copy,
                            bias=-8.0, scale=1.0)
                    else:  # "dve", "dve_pool"
                        sub_eng = nc.vector if dq_mode == "dve" else nc.gpsimd
                        sub_eng.tensor_scalar(
                            out=lo[:, :nch], in0=lq[:, :nch], scalar1=8,
                            scalar2=None, op0=mybir.AluOpType.subtract)
                        sub_eng.tensor_scalar(
                            out=hi[:, :nch], in0=hq[:, :nch], scalar1=8,
                            scalar2=None, op0=mybir.AluOpType.subtract)
                    mult2_eng = (nc.gpsimd if dq_mode in ("dve_pool", "dve_act_pool")
                                 else nc.vector)
                    nc.vector.tensor_tensor(
                        out=wsb[:, 2 * b, n0:n0 + nch], in0=lo[:, :nch],
                        in1=st[:, b, :nch], op=mybir.AluOpType.mult)
                    mult2_eng.tensor_tensor(
                        out=wsb[:, 2 * b + 1, n0:n0 + nch], in0=hi[:, :nch],
                        in1=st[:, b, :nch], op=mybir.AluOpType.mult)

            # ---- main loop: groups of m-chunks share one wide At DMA set ----
            for g in range(0 if _skip_main else mg):
                at4 = atpool.tile([P, NKB, MW], mybir.dt.bfloat16)
                # whole group in ONE strided DMA.  The sync ring only issues
                # DMAs (sub-us each), so this is never queued behind compute;
                # on the ACT ring it would sit behind all 96 dequant subs
                # (in-order queue) and stall the PE's first group ~60us.
                nc.sync.dma_start(
                    out=at4, in_=AT_V[:, :, g * MW:(g + 1) * MW])
                for mci in range(mpg):
                    mc = g * mpg + mci
                    o_fin = opool.tile([P, ns], mybir.dt.float32, tag="of")
                    for (n0, nch) in n_chunks:
                        po = psum_o.tile([P, 512], mybir.dt.float32, tag="po")
                        for kb in range(NKB):
                            nc.tensor.matmul(
                                po[:, :nch],
                                lhsT=at4[:, kb, mci * P:(mci + 1) * P],
                                rhs=wsb[:, kb, n0:n0 + nch],
                                start=(kb == 0), stop=(kb == NKB - 1))
                        nc.vector.tensor_tensor(
                            out=o_fin[:, n0:n0 + nch], in0=po[:, :nch],
                            in1=bias_t[:, n0:n0 + nch], op=mybir.AluOpType.add)
                    nc.sync.dma_start(
                        out=OUT[mc * P:(mc + 1) * P, :], in_=o_fin)

          if reps == 1:
              body()
          else:
              # hardware loop: constant program size for any rep count —
              # used only by the timing harness
              with tc.For_i(0, reps):
                  body()

    # Bacc.finalize() runs compile() (register allocation + sync legalization)
    # and then freezes the module for the bass_exec PJRT path.
    nc.finalize()
    return nc


_NC_CACHE = {}


def _get_nc():
    if "nc" not in _NC_CACHE:
        _NC_CACHE["nc"] = build_nc()
    return _NC_CACHE["nc"]


def shard_inputs(A, weight, weight_scale, bias):
    A = np.asarray(A, dtype=np.float32)
    # kb-major k-on-partitions A^T in bf16: At[kb, p, m] = A[m, k] with
    # k = 256*(kb//2) + 2p + (kb%2) — the k order the nibble unpack
    # produces.  One host transpose shared by all 8 cores.
    At = np.ascontiguousarray(
        A.astype(bfloat16).T.reshape(NB2, P, 2, M).transpose(0, 2, 1, 3)
        .reshape(NKB, P, M))
    wq = np.asarray(weight, dtype=np.uint8)
    ws = np.asarray(weight_scale, dtype=np.float32)
    bs = np.asarray(bias, dtype=np.float32)
    in_maps = []
    for c in range(NCORES):
        sl = slice(c * NS, (c + 1) * NS)
        in_maps.append({
            "at": At,
            "wq": np.ascontiguousarray(wq[:, sl]),
            # replicate each scale row 16x so row k2 of srep carries the
            # scale for packed row k2 (group g = k2 // 16); bf16 halves DMA
            "srep": np.ascontiguousarray(
                np.repeat(ws[:, sl], 16, axis=0).astype(bfloat16)),
            # partition-replicated so the device DMA is a plain 2D copy
            "bias": np.ascontiguousarray(np.broadcast_to(bs[sl], (P, NS))),
        })
    return in_maps


def run(inputs, trace=False, **kw):
    nc = _get_nc()
    in_maps = shard_inputs(**inputs)
    res = run_bass_kernel_spmd(nc, in_maps, core_ids=list(range(NCORES)), trace=trace, **kw)
    out = np.concatenate([res.results[c]["out"] for c in range(NCORES)], axis=1)
    return out, res


def kernel(A, weight, weight_scale, bias):
    out, _ = run(dict(A=A, weight=weight, weight_scale=weight_scale, bias=bias))
    return out
